# revision 24
# baseline (speedup 1.0000x reference)
"""Exact top-k (k=32) attention on 8 Trainium2 NeuronCores.

Strategy (head-parallel + key compaction): the 16 (batch, head) pairs are
sharded 2-per-core; core i gets (n=0, h=i) and (n=1, h=i), so every core
holds one head of each batch item and the per-core work is identical even
though the two batch items have different key_lengths.

Key compaction: keys beyond key_lengths[n] can never be selected (the
reference masks them to -inf), so only the first klen_n keys are shipped,
permuted, and padded up to C_n = ceil(klen_n/128) chunks of 128.  All
s-proportional work (score matmuls, selection scans, exp/sign, AV) runs
over C_n chunks instead of S/128 = 16.  The bass program is built on the
first kernel() call from the actual key_lengths (cached per (C0, C1)).

Per head, per core:
  Phase 1 (selection): forward scores F[q, s] via a 2-pass bf16-split matmul
    (hi*hi + partial lo*lo folded into pass A; hi*lo + lo*hi in pass BC;
    accurate to ~1e-5); candidate top-8 of each 128-wide key chunk read
    straight from PSUM by C narrow DVE max8 ops, then the 32nd-largest of
    the 8C candidates via 4 rounds of max8 / match_replace.  This equals
    the row's exact 32nd-largest unless one chunk holds >= 9 of the row's
    top-32; such rows make the on-device selection count exceed 32 and are
    recomputed on the host (vectorized) like tie rows.  Cut value
    t_minus = t - |t|*2^-19 - 1e-37, strictly inside (s_33, s_32].
  Phase 2 (apply): transposed scores minus t_minus computed directly by an
    augmented matmul (extra contraction rows: klen mask x ones, ones x
    (-t1,-t2,-t3) with t decomposed into 3 bf16 terms), giving
    d'[s, q] = scores^T - t_minus in PSUM (bit-identical products to the
    forward pass).  Then
      g = Exp(temp*d')        (ScalarE, bf16)
      S = Sign(d')            (ScalarE, bf16, in {-1,+1})
      A' = max(g - 1, 0)      (GPSIMD, bf16) == (w - 1) on selected, 0 off
    and AV is reconstructed via
      sum_sel w*V = V^T A' + 0.5*(V^T S + sum_s V)
    using an appended ones-column of V to carry Z = sum_sel w and the
    selection count.
  Phase 1 runs as a free-running stream ~1 subtile per phase-2 chunk,
  decoupled from the slot boundaries, so each slot's cut values are staged
  well before its phase 2 begins and the PE never goes idle at a slot
  boundary (HAM stays warm).
  A per-row selection count is returned; rows where it is not exactly 32
  (candidate-segment overflow, or s_33 within ~2^-19*|t| of s_32) are
  recomputed exactly on the host with a vectorized numpy path.
"""

import numpy as np
import ml_dtypes

N, L, S, H, E, D = 2, 2048, 2048, 8, 64, 64
TOPK = 32
TEMP = 1.0 / np.sqrt(E)
HEADS_PER_CORE = 2
N_CORES = 8
LT = 16          # L tiles of 128
QB = 4           # q blocks of 512 in phase 2
NEG = -1e30
NLO = 60         # e-rows of the lo*lo partial correction in pass A
AP_ON_GPSIMD = False  # Q7 tensor_scalar measured ~7.4us per [128,512] op --
                      # far below line rate; keep A' = relu(g-1) on the DVE

_bf16 = ml_dtypes.bfloat16


def _chunks_for(klen):
    return (int(klen) + 127) // 128


def _slot_order(CS):
    """(hh, g) slots in schedule order: larger head first, so the slots
    left without phase-1 overlap at the end are the cheap ones."""
    heads = sorted(range(HEADS_PER_CORE), key=lambda hh: -CS[hh])
    return [(hh, g) for hh in heads for g in range(QB)]


def _ind_groups(CS):
    """Slots run in indicator mode (no Sign; count = sum of 1{d'>0}):
    the final two slots, whose phase 2 has no phase-1 stream left to
    overlap and would otherwise pace on the ScalarE exp+sign pair."""
    return set(_slot_order(CS)[-2:])


def _build_bass(CS):
    """CS: tuple of per-head chunk counts, e.g. (10, 12)."""
    import concourse.mybir as mybir
    from concourse import bacc
    from concourse.tile import TileContext
    from concourse.masks import make_identity
    from collections import deque

    f32 = mybir.dt.float32
    bf16 = mybir.dt.bfloat16

    nc = bacc.Bacc()
    HPC = HEADS_PER_CORE
    assert len(CS) == HPC
    SH = [c * 128 for c in CS]          # padded key count per head
    NS = [(c + 3) // 4 for c in CS]     # 512-wide pf subtiles per L-tile

    qa_d, ka_d, qbc_d, kbc_d, va_d, out_d, nsel_d = [], [], [], [], [], [], []
    for hh in range(HPC):
        qa_d.append(nc.declare_dram_parameter(f"qa{hh}", [128, L], bf16,
                                              isOutput=False))
        ka_d.append(nc.declare_dram_parameter(f"ka{hh}", [128, SH[hh]], bf16,
                                              isOutput=False))
        qbc_d.append(nc.declare_dram_parameter(f"qbc{hh}", [128, L], bf16,
                                               isOutput=False))
        kbc_d.append(nc.declare_dram_parameter(f"kbc{hh}", [128, SH[hh]],
                                               bf16, isOutput=False))
        va_d.append(nc.declare_dram_parameter(f"va{hh}", [CS[hh], 128, D + 1],
                                              bf16, isOutput=False))
        out_d.append(nc.declare_dram_parameter(f"out{hh}", [L, D], f32,
                                               isOutput=True))
        nsel_d.append(nc.declare_dram_parameter(f"nsel{hh}", [L], f32,
                                                isOutput=True))

    from contextlib import ExitStack
    with TileContext(nc) as tc, ExitStack() as ctx:
        consts = ctx.enter_context(tc.tile_pool(name="consts", bufs=1))
        inpool = ctx.enter_context(tc.tile_pool(name="inputs", bufs=1))
        cpool = ctx.enter_context(tc.tile_pool(name="cands", bufs=4))
        small = ctx.enter_context(tc.tile_pool(name="small", bufs=3))
        gs_pool = ctx.enter_context(tc.tile_pool(name="gs", bufs=4))
        opool = ctx.enter_context(tc.tile_pool(name="outbuf", bufs=3))
        ps_f = ctx.enter_context(tc.tile_pool(name="ps_fwd", bufs=2, space="PSUM"))
        ps_t = ctx.enter_context(tc.tile_pool(name="ps_t", bufs=2, space="PSUM"))
        ps_av = ctx.enter_context(tc.tile_pool(name="ps_av", bufs=1, space="PSUM"))
        ps_x = ctx.enter_context(tc.tile_pool(name="ps_x", bufs=1, space="PSUM"))

        ident = consts.tile([128, 128], bf16)
        make_identity(nc, ident)
        ident32 = consts.tile([128, 128], f32)
        make_identity(nc, ident32)
        ones_col = consts.tile([128, 1], bf16)
        nc.vector.memset(ones_col, 1.0)

        # ---- load all inputs; head-0 p1 operands first (they gate the
        # prologue), spread across queues so dispatch doesn't serialize ----
        qa = [None] * HPC
        ka = [None] * HPC
        qbc = [None] * HPC
        kbc = [None] * HPC
        va = [None] * HPC
        for hh in range(HPC):
            qa[hh] = inpool.tile([128, L], bf16, tag=f"qa{hh}", name=f"qa{hh}")
            ka[hh] = inpool.tile([128, SH[hh]], bf16, tag=f"ka{hh}",
                                 name=f"ka{hh}")
            qbc[hh] = inpool.tile([128, L], bf16, tag=f"qbc{hh}",
                                  name=f"qbc{hh}")
            kbc[hh] = inpool.tile([128, SH[hh]], bf16, tag=f"kbc{hh}",
                                  name=f"kbc{hh}")
            va[hh] = inpool.tile([128, CS[hh], D + 1], bf16, tag=f"va{hh}",
                                 name=f"va{hh}")
        slot_order = _slot_order(CS)
        ind_groups = _ind_groups(CS)
        h1 = slot_order[0][0]      # head scheduled first (prologue head)
        h2 = 1 - h1
        # first pieces cover the prologue's operands so phase 1 starts
        # within a couple of microseconds of kernel start
        nc.sync.dma_start(ka[h1][:, 0:512], ka_d[h1][:, 0:512])
        nc.sync.dma_start(qa[h1][:, 0:512], qa_d[h1][:, 0:512])
        nc.scalar.dma_start(kbc[h1][:, 0:512], kbc_d[h1][:, 0:512])
        nc.scalar.dma_start(qbc[h1][:, 0:512], qbc_d[h1][:, 0:512])
        nc.sync.dma_start(ka[h1][:, 512:SH[h1]], ka_d[h1][:, 512:SH[h1]])
        nc.sync.dma_start(qa[h1][:, 512:L], qa_d[h1][:, 512:L])
        nc.scalar.dma_start(kbc[h1][:, 512:SH[h1]], kbc_d[h1][:, 512:SH[h1]])
        nc.scalar.dma_start(qbc[h1][:, 512:L], qbc_d[h1][:, 512:L])
        nc.sync.dma_start(qa[h2], qa_d[h2][:, :])
        nc.sync.dma_start(ka[h2], ka_d[h2][:, :])
        nc.scalar.dma_start(qbc[h2], qbc_d[h2][:, :])
        nc.scalar.dma_start(kbc[h2], kbc_d[h2][:, :])
        nc.gpsimd.dma_start(va[h1], va_d[h1].rearrange("c p d -> p c d"))
        nc.gpsimd.dma_start(va[h2], va_d[h2].rearrange("c p d -> p c d"))

        halfsum = [None] * HPC

        def head_prep(hh):
            # 0.5 * sum_s V_aug
            pv = ps_x.tile([128, 128], f32, tag="tpose", name="tpose")
            for c in range(CS[hh]):
                nc.tensor.matmul(pv[0:D + 1, 0:1], va[hh][:, c, :], ones_col,
                                 start=(c == 0), stop=(c == CS[hh] - 1))
            halfsum[hh] = small.tile([D + 1, 1], f32, tag=f"halfsum{hh}",
                                     name=f"halfsum{hh}")
            nc.scalar.activation(halfsum[hh], pv[0:D + 1, 0:1],
                                 mybir.ActivationFunctionType.Copy, scale=0.5)

        def p1_pf(hh, lt, q4, pool=None, tag="fwd"):
            """forward scores for up to 512 keys of tile lt."""
            w = min(512, SH[hh] - q4 * 512)
            pf = (pool or ps_f).tile([128, 512], f32, tag=tag, name=tag)
            nc.tensor.matmul(pf[:, 0:w], qa[hh][:, lt * 128:(lt + 1) * 128],
                             ka[hh][:, q4 * 512:q4 * 512 + w],
                             start=True, stop=False)
            nc.tensor.matmul(pf[:, 0:w], qbc[hh][:, lt * 128:(lt + 1) * 128],
                             kbc[hh][:, q4 * 512:q4 * 512 + w],
                             start=False, stop=True)
            return pf

        def p1_max8(hh, pf, q4, cands):
            """top-8 of each 128-wide chunk, straight from PSUM."""
            nch = min(4, CS[hh] - 4 * q4)
            for j in range(nch):
                c0 = (4 * q4 + j) * 8
                nc.vector.max(out=cands[:, c0:c0 + 8],
                              in_=pf[:, j * 128:(j + 1) * 128])

        def p1_extract_a(cands, m_sb):
            """extraction rounds 0-1 (max8, mr, max8, mr)."""
            for r in range(2):
                nc.vector.max(out=m_sb[:, 8 * r:8 * r + 8], in_=cands)
                nc.vector.match_replace(
                    out=cands, in_to_replace=m_sb[:, 8 * r:8 * r + 8],
                    in_values=cands, imm_value=NEG)

        def p1_extract_b(i, cands, m_sb, tcol4):
            """extraction rounds 2-3; t32 -> tcol4 col i."""
            nc.vector.max(out=m_sb[:, 16:24], in_=cands)
            nc.vector.match_replace(
                out=cands, in_to_replace=m_sb[:, 16:24],
                in_values=cands, imm_value=NEG)
            nc.vector.max(out=m_sb[:, 24:32], in_=cands)
            nc.vector.tensor_copy(tcol4[:, i:i + 1], m_sb[:, 31:32])

        def p1_split(tcols, tcol4):
            """batched t_minus + bf16 triple split for the 4 tiles.

            m = -(t - |t|*2^-19 - 1e-37) = |t|*2^-19 + 1e-37 - t
            (2^-19, not 1 ulp: phase 2 folds -t into the accumulation
            before the lo-product rows, so its rounding path differs
            from phase 1's by ~±8e-6; the cut needs to clear that.)"""
            acol = small.tile([128, 12], f32, tag="tm", name="tm")
            nc.scalar.activation(acol[:, 0:4], tcol4,
                                 mybir.ActivationFunctionType.Abs,
                                 scale=float(2.0 ** -19))
            nc.vector.scalar_tensor_tensor(
                out=acol[:, 4:8], in0=acol[:, 0:4], scalar=1e-37, in1=tcol4,
                op0=mybir.AluOpType.add, op1=mybir.AluOpType.subtract)
            nc.vector.tensor_copy(tcols[:, 0:4], acol[:, 4:8])
            nc.vector.tensor_tensor(
                out=acol[:, 8:12], in0=acol[:, 4:8], in1=tcols[:, 0:4],
                op=mybir.AluOpType.subtract)
            nc.vector.tensor_copy(tcols[:, 4:8], acol[:, 8:12])
            nc.vector.tensor_tensor(
                out=acol[:, 0:4], in0=acol[:, 8:12], in1=tcols[:, 4:8],
                op=mybir.AluOpType.subtract)
            nc.vector.tensor_copy(tcols[:, 8:12], acol[:, 0:4])

        def p1_stage(hh, g, tcols):
            """transpose tcols into qa rows 65..67, cols of q-group g."""
            pt = ps_x.tile([128, 128], bf16, tag="tposeb", name="tposeb")
            nc.tensor.transpose(pt[0:12, :], tcols, ident)
            stage = small.tile([12, 128], bf16, tag="stage12", name="stage12")
            nc.scalar.copy(out=stage, in_=pt[0:12, :])
            nc.sync.dma_start(
                qa[hh][65:68, g * 512:(g + 1) * 512].rearrange(
                    "p (t q) -> p t q", t=4),
                stage[:, :])

        def p2_pt(hh, g, c):
            qs = slice(g * 512, (g + 1) * 512)
            pt = ps_t.tile([128, 512], f32, tag="psumT", name="psumT")
            nc.tensor.matmul(pt, ka[hh][:, c * 128:(c + 1) * 128],
                             qa[hh][:, qs], start=True, stop=False)
            nc.tensor.matmul(pt, kbc[hh][:, c * 128:(c + 1) * 128],
                             qbc[hh][:, qs], start=False, stop=True)
            return pt

        def p2_act(pt, want_sign):
            g_sb = gs_pool.tile([128, 512], bf16, tag="g", name="g")
            nc.scalar.activation(g_sb, pt,
                                 mybir.ActivationFunctionType.Exp,
                                 scale=float(TEMP))
            if not want_sign:
                return g_sb, None
            s_sb = gs_pool.tile([128, 512], bf16, tag="s", name="s")
            nc.scalar.activation(s_sb, pt,
                                 mybir.ActivationFunctionType.Sign)
            return g_sb, s_sb

        def p2_ind(pt):
            """ind = 1{d' > 0} straight from PSUM (DVE; exact cut -- the
            32nd key sits only ~|t|*2^-19 above t_minus, so the compare
            must happen on the fp32 scores, not on bf16 g)."""
            ind_sb = gs_pool.tile([128, 512], bf16, tag="s", name="ind")
            nc.vector.tensor_scalar(
                out=ind_sb, in0=pt, scalar1=0.0, scalar2=None,
                op0=mybir.AluOpType.is_gt)
            return ind_sb

        def p2_ap(g_sb):
            ap_sb = gs_pool.tile([128, 512], bf16, tag="ap", name="ap")
            eng = nc.gpsimd if AP_ON_GPSIMD else nc.vector
            eng.tensor_scalar(
                out=ap_sb, in0=g_sb, scalar1=1.0, scalar2=0.0,
                op0=mybir.AluOpType.subtract, op1=mybir.AluOpType.max)
            return ap_sb

        def p2_av(hh, c, av_g, av_s, ap_sb, s_sb):
            nc.tensor.matmul(av_g, va[hh][:, c, :], ap_sb,
                             start=(c == 0), stop=(c == CS[hh] - 1))
            nc.tensor.matmul(av_s, va[hh][:, c, :], s_sb,
                             start=(c == 0), stop=(c == CS[hh] - 1))

        def p2_tail_u(hh, g, av_g, av_s):
            # selection count -> host.  Sign mode: row D of av_s is
            # 2*cnt - SH; indicator mode: row D of av_ind is cnt.
            is_ind = (hh, g) in ind_groups
            nsel_sb = opool.tile([1, 512], f32, tag="nsel", name="nsel")
            nc.scalar.copy(out=nsel_sb, in_=av_s[D:D + 1, :])
            nc.scalar.dma_start(nsel_d[hh][g * 512:(g + 1) * 512], nsel_sb)
            # sign mode:      u = (0.5*av_s + halfsum) + av_g
            # indicator mode: u = (1.0*av_ind + 0)     + av_g
            u1_sb = opool.tile([D + 1, 512], f32, tag="u1", name="u1")
            nc.scalar.activation(u1_sb, av_s[0:D + 1, :],
                                 mybir.ActivationFunctionType.Identity,
                                 bias=0.0 if is_ind else halfsum[hh],
                                 scale=1.0 if is_ind else 0.5)
            u_sb = opool.tile([D + 1, 512], f32, tag="u", name="u")
            nc.vector.tensor_tensor(out=u_sb, in0=u1_sb, in1=av_g,
                                    op=mybir.AluOpType.add)
            return u_sb

        def p2_tail_out(hh, g, u_sb, sub):
            po = ps_x.tile([128, 128], f32, tag="tpose", name="tpose")
            nc.tensor.transpose(po[:, 0:D + 1],
                                u_sb[:, sub * 128:(sub + 1) * 128],
                                ident32[0:D + 1, 0:D + 1])
            recip = opool.tile([128, 1], f32, tag="recip", name="recip")
            nc.vector.reciprocal(out=recip, in_=po[:, D:D + 1])
            o_sb = opool.tile([128, D], f32, tag="osb", name="osb")
            nc.vector.tensor_scalar(
                out=o_sb, in0=po[:, 0:D], scalar1=recip, scalar2=None,
                op0=mybir.AluOpType.mult)
            lq = g * 512 + sub * 128
            nc.scalar.dma_start(out_d[hh][lq:lq + 128, :], o_sb)

        # ---- pipeline ----
        # slots = (head, q-group); slot s's phase 2 streams its C chunks
        # while the free-running phase-1 stream (1 subtile per step) works
        # ~1.5 slots ahead, so cut values are staged early.
        slots = slot_order

        # global p1 stream state
        p1q = deque()            # (sidx, hh, g, tile_i, q4)
        for sidx in range(1, len(slots)):
            hh, g = slots[sidx]
            for i in range(4):
                for q4 in range(NS[hh]):
                    p1q.append((sidx, hh, g, i, q4))
        sstate = {}              # sidx -> dict(cands, msb, tcol4, tcols, nx)
        ext_q = deque()          # ('a'|'b', sidx, i)
        pending_pf = None        # (sidx, hh, i, q4, pf) awaiting max8

        def p1_state(sidx, hh, g):
            st = sstate.get(sidx)
            if st is None:
                st = dict(cands={}, msb={}, extracted=0,
                          tcol4=small.tile([128, 4], f32, tag="t4",
                                           name="t4"))
                sstate[sidx] = st
            return st

        def p1_step_issue():
            """issue next pf of the global stream (PE)."""
            nonlocal pending_pf
            if not p1q:
                return
            sidx, hh, g, i, q4 = p1q.popleft()
            st = p1_state(sidx, hh, g)
            if q4 == 0:
                st['cands'][i] = cpool.tile([128, 8 * CS[hh]], f32,
                                            tag=f"cands{hh}", name="cands")
            pf = p1_pf(hh, 4 * g + i, q4)
            pending_pf = (sidx, hh, g, i, q4, pf)

        def p1_step_max8():
            """emit max8s for the pf issued last step (DVE)."""
            nonlocal pending_pf
            if pending_pf is None:
                return
            sidx, hh, g, i, q4, pf = pending_pf
            pending_pf = None
            st = sstate[sidx]
            p1_max8(hh, pf, q4, st['cands'][i])
            if q4 == NS[hh] - 1:
                ext_q.append(('a', sidx, hh, g, i))
                ext_q.append(('b', sidx, hh, g, i))

        def p1_step_extract(budget=2):
            """drain up to `budget` extraction stages (DVE)."""
            while budget > 0 and ext_q:
                kind, sidx, hh, g, i = ext_q.popleft()
                st = sstate[sidx]
                if kind == 'a':
                    st['msb'][i] = small.tile([128, 32], f32, tag="m32",
                                              name="m32")
                    p1_extract_a(st['cands'][i], st['msb'][i])
                else:
                    p1_extract_b(i, st['cands'][i], st['msb'][i],
                                 st['tcol4'])
                    del st['cands'][i]
                    del st['msb'][i]
                    st['extracted'] += 1
                    if st['extracted'] == 4:
                        tcols = small.tile([128, 12], bf16, tag="tcols",
                                           name="tcols")
                        p1_split(tcols, st['tcol4'])
                        p1_stage(hh, g, tcols)
                        del sstate[sidx]
                budget -= 1

        def p1_whole(hh, g, tcols0):
            """prologue phase 1 for slot 0: pf ring borrows the idle ps_t
            bank pair (4-deep pipeline) and max8s trail their pf by one
            subtile so the PE never waits on the DVE."""
            ns = NS[hh]
            nsub = 4 * ns
            tcol4 = small.tile([128, 4], f32, tag="t4", name="t4")
            candss = {}
            msbs = {}
            pfs = {}
            for sc in range(nsub + 3):
                if sc < nsub:
                    i, q4 = sc // ns, sc % ns
                    if q4 == 0:
                        candss[i] = cpool.tile([128, 8 * CS[hh]], f32,
                                               tag=f"cands{hh}",
                                               name="cands")
                    pool, tag = ((ps_f, "fwd") if sc % 2 == 0
                                 else (ps_t, "psumT"))
                    pfs[sc] = p1_pf(hh, 4 * g + i, q4, pool, tag)
                s1 = sc - 1
                if 0 <= s1 < nsub:
                    p1_max8(hh, pfs[s1], s1 % ns, candss[s1 // ns])
                    del pfs[s1]
                s2 = sc - ns - 1
                if s2 >= 0 and s2 % ns == 0 and s2 // ns < 4:
                    i = s2 // ns
                    msbs[i] = small.tile([128, 32], f32, tag="m32",
                                         name="m32")
                    p1_extract_a(candss[i], msbs[i])
                s3 = sc - ns - 2
                if s3 >= 0 and s3 % ns == 0 and s3 // ns < 4:
                    i = s3 // ns
                    p1_extract_b(i, candss[i], msbs[i], tcol4)
            p1_split(tcols0, tcol4)
            p1_stage(hh, g, tcols0)

        tcols0 = small.tile([128, 12], bf16, tag="tcols", name="tcols")
        p1_whole(*slots[0], tcols0)
        # after the prologue: their matmuls wait on the va DMAs, and at the
        # head of the PE queue they would stall the prologue's score MMs
        head_prep(0)
        head_prep(1)

        prev_tail = None   # (hh, g, av_g, av_s) of the previous slot
        prev_u = None
        for s, (hh, g) in enumerate(slots):
            C = CS[hh]
            is_ind = (hh, g) in ind_groups
            av_g = ps_av.tile([D + 1, 512], f32, tag="av_g", name="av_g")
            av_s = ps_av.tile([D + 1, 512], f32, tag="av_s", name="av_s")
            pts = {}
            gss = {}
            aps = {}
            for step in range(C + 3):
                c, c1, c2 = step, step - 1, step - 2
                # DVE: ap for chunk c2 first (exp finished a step ago), so
                # the av matmuls emitted below stall minimally
                if 0 <= c2 < C:
                    aps[c2] = p2_ap(gss[c2][0])
                # PE: pt(c), then av(c2), then the p1-stream pf
                if c < C:
                    pts[c] = p2_pt(hh, g, c)
                if 0 <= c2 < C:
                    p2_av(hh, c2, av_g, av_s, aps[c2], gss[c2][1])
                    del aps[c2]
                p1_step_issue()
                # previous slot's tail, spread over this slot's first steps
                if prev_tail is not None:
                    ph, pg, pav_g, pav_s = prev_tail
                    if step == 0:
                        prev_u = p2_tail_u(ph, pg, pav_g, pav_s)
                    elif step <= 4:
                        p2_tail_out(ph, pg, prev_u, step - 1)
                        if step == 4:
                            prev_tail = None
                # DVE: chunk-max8s of the pf issued last step, then up to
                # two extraction stages
                p1_step_max8()
                p1_step_extract(2)
                # ScalarE: exp (+ sign unless indicator mode) of pt(c1);
                # indicator mode takes 1{d'>0} from PSUM on the DVE while
                # pt(c1) is still live
                if 0 <= c1 < C:
                    g_sb, s_sb = p2_act(pts[c1], want_sign=not is_ind)
                    if is_ind:
                        s_sb = p2_ind(pts[c1])
                    gss[c1] = (g_sb, s_sb)
                    del pts[c1]
            prev_tail = (hh, g, av_g, av_s)
        # drain any remaining p1 work (shouldn't happen) and final tail
        while p1q or ext_q or pending_pf is not None:
            p1_step_issue()
            p1_step_max8()
            p1_step_extract(4)
        ph, pg, pav_g, pav_s = prev_tail
        u_last = p2_tail_u(ph, pg, pav_g, pav_s)
        for sub in range(4):
            p2_tail_out(ph, pg, u_last, sub)

    nc.compile()
    return nc


_NC_CACHE = {}


def _get_nc(CS):
    key = tuple(CS)
    if key not in _NC_CACHE:
        _NC_CACHE[key] = _build_bass(key)
    return _NC_CACHE[key]


def _split_hi_lo(x):
    hi = x.astype(_bf16)
    lo = (x.astype(np.float32) - hi.astype(np.float32)).astype(_bf16)
    return hi, lo


def _host_fix_rows(out, fix_rows, queries, keys, values, key_lengths):
    """Vectorized exact fp32 recompute of rows whose selection count != 32."""
    by_nh = {}
    for (n, lq, h) in fix_rows:
        by_nh.setdefault((n, h), []).append(lq)
    for (n, h), lqs in by_nh.items():
        lqs = np.asarray(sorted(lqs))
        kl = int(key_lengths[n])
        Q = np.asarray(queries[n, lqs, h, :], np.float32)      # [m, E]
        K = np.asarray(keys[n, :kl, h, :], np.float32)         # [kl, E]
        V = np.asarray(values[n, :kl, h, :], np.float32)       # [kl, D]
        s = Q @ K.T                                            # [m, kl]
        idx = np.argpartition(-s, TOPK - 1, axis=1)[:, :TOPK]
        sv = np.take_along_axis(s, idx, axis=1)
        w = np.exp(TEMP * (sv - sv.max(axis=1, keepdims=True)))
        w /= w.sum(axis=1, keepdims=True)
        out[n, lqs, h, :] = np.einsum('mk,mkd->md', w, V[idx])


def _key_perm(n, klen):
    """Fixed random permutation of the klen valid key slots of batch n.

    The output is invariant to key order, but the candidate-chunk
    prefilter assumes the top-32 positions of a row are spread across
    chunks; scattering valid keys over the compacted range gives every
    chunk ~klen/C valid keys and the balls-in-bins behaviour (rows where
    a chunk overflows are caught by the count check and host-fixed)."""
    rng = np.random.default_rng(1234567 + n)
    return rng.permutation(klen)


def _prep_core(core, queries, keys, values, key_lengths_i):
    pairs = [(0, core), (1, core)]
    CS = [_chunks_for(key_lengths_i[n]) for n, _ in pairs]
    im = {}
    for i, (n, h) in enumerate(pairs):
        klen = int(key_lengths_i[n])
        C = CS[i]
        Sh = C * 128
        perm = _key_perm(n, klen)
        qa = np.zeros((128, L), _bf16)
        ka = np.zeros((128, Sh), _bf16)
        qbc = np.zeros((128, L), _bf16)
        kbc = np.zeros((128, Sh), _bf16)
        va = np.zeros((C, 128, D + 1), _bf16)
        Q = queries[n, :, h, :]                      # [L, E]
        K = np.zeros((Sh, E), np.float32)
        V = np.zeros((Sh, D), np.float32)
        K[:klen] = keys[n, :klen, h, :][perm]
        V[:klen] = values[n, :klen, h, :][perm]
        qh, ql = _split_hi_lo(Q)
        kh, kl_ = _split_hi_lo(K)
        mask = np.full(Sh, NEG, np.float32)
        mask[:klen] = 0.0
        qa[0:E, :] = qh.T
        qa[E, :] = 1.0
        # rows 65..67 stay 0 (t slots, filled on device)
        qa[E + 4:E + 4 + NLO, :] = ql.T[0:NLO]
        ka[0:E, :] = kh.T
        ka[E, :] = mask.astype(_bf16)
        ka[E + 1:E + 4, :] = 1.0
        ka[E + 4:E + 4 + NLO, :] = kl_.T[0:NLO]
        qbc[0:E, :] = qh.T
        qbc[E:2 * E, :] = ql.T
        kbc[0:E, :] = kl_.T
        kbc[E:2 * E, :] = kh.T
        va[:, :, 0:D] = V.astype(_bf16).reshape(C, 128, D)
        va[:, :, D] = 1.0
        im.update({f"qa{i}": qa, f"ka{i}": ka, f"qbc{i}": qbc,
                   f"kbc{i}": kbc, f"va{i}": va})
    return pairs, CS, im


def kernel(queries, keys, values, key_lengths):
    from concourse.bass_utils import run_bass_kernel_spmd

    queries = np.asarray(queries, np.float32)
    keys = np.asarray(keys, np.float32)
    values = np.asarray(values, np.float32)
    key_lengths_i = np.asarray(key_lengths).astype(np.int64)

    in_maps = []
    head_map = []  # per core: list of (n, h)
    CS = None
    for core in range(N_CORES):
        pairs, CS, im = _prep_core(core, queries, keys, values,
                                   key_lengths_i)
        head_map.append(pairs)
        in_maps.append(im)

    nc = _get_nc(CS)
    res = run_bass_kernel_spmd(nc, in_maps, list(range(N_CORES)))

    out = np.zeros((N, L, H, D), np.float32)
    ind_groups = _ind_groups(CS)
    fix_rows = []
    for core in range(N_CORES):
        for i, (n, h) in enumerate(head_map[core]):
            o = res.results[core][f"out{i}"].reshape(L, D)
            nsel = res.results[core][f"nsel{i}"].reshape(L)
            out[n, :, h, :] = o
            cnt = (nsel + CS[i] * 128) * 0.5
            for g in range(QB):
                if (i, g) in ind_groups:
                    cnt[g * 512:(g + 1) * 512] = nsel[g * 512:(g + 1) * 512]
            bad = np.nonzero(cnt != TOPK)[0]
            for lq in bad:
                fix_rows.append((n, int(lq), h))
    if fix_rows:
        _host_fix_rows(out, fix_rows, queries, keys, values, key_lengths_i)
    return out


# revision 30
# speedup vs baseline: 1.0705x; 1.0705x over previous
"""Exact top-k (k=32) attention on 8 Trainium2 NeuronCores.

Strategy (head-parallel + key compaction): the 16 (batch, head) pairs are
sharded 2-per-core; core i gets (n=0, h=i) and (n=1, h=i), so every core
holds one head of each batch item and the per-core work is identical even
though the two batch items have different key_lengths.

Key compaction: keys beyond key_lengths[n] can never be selected (the
reference masks them to -inf), so only the first klen_n keys are shipped,
permuted, and padded up to C_n = ceil(klen_n/128) chunks of 128.  All
s-proportional work (score matmuls, selection scans, exp/sign, AV) runs
over C_n chunks instead of S/128 = 16.  The bass program is built on the
first kernel() call from the actual key_lengths (cached per (C0, C1)).

Per head, per core:
  Phase 1 (selection): forward scores F[q, s] via a 2-pass bf16-split matmul
    (hi*hi + partial lo*lo folded into pass A; hi*lo + lo*hi in pass BC;
    accurate to ~1e-5); candidate top-8 of each 128-wide key chunk read
    straight from PSUM by C narrow DVE max8 ops, then the 32nd-largest of
    the 8C candidates via 4 rounds of max8 / match_replace.  This equals
    the row's exact 32nd-largest unless one chunk holds >= 9 of the row's
    top-32; such rows make the on-device selection count exceed 32 and are
    recomputed on the host (vectorized) like tie rows.  Cut value
    t_minus = t - |t|*2^-19 - 1e-37, strictly inside (s_33, s_32].
  Phase 2 (apply): transposed scores minus t_minus computed directly by an
    augmented matmul (extra contraction rows: klen mask x ones, ones x
    (-t1,-t2,-t3) with t decomposed into 3 bf16 terms), giving
    d'[s, q] = scores^T - t_minus in PSUM (bit-identical products to the
    forward pass).  Then
      g = Exp(temp*d')        (ScalarE, bf16)
      S = Sign(d')            (ScalarE, bf16, in {-1,+1})
      A' = max(g - 1, 0)      (GPSIMD, bf16) == (w - 1) on selected, 0 off
    and AV is reconstructed via
      sum_sel w*V = V^T A' + 0.5*(V^T S + sum_s V)
    using an appended ones-column of V to carry Z = sum_sel w and the
    selection count.
  Phase 1 runs as a free-running stream ~1 subtile per phase-2 chunk,
  decoupled from the slot boundaries, so each slot's cut values are staged
  well before its phase 2 begins and the PE never goes idle at a slot
  boundary (HAM stays warm).
  A per-row selection count is returned; rows where it is not exactly 32
  (candidate-segment overflow, or s_33 within ~2^-19*|t| of s_32) are
  recomputed exactly on the host with a vectorized numpy path.
"""

import numpy as np
import ml_dtypes

N, L, S, H, E, D = 2, 2048, 2048, 8, 64, 64
TOPK = 32
TEMP = 1.0 / np.sqrt(E)
HEADS_PER_CORE = 2
N_CORES = 8
LT = 16          # L tiles of 128
QB = 4           # q blocks of 512 in phase 2
NEG = -1e30
NLO = 60         # e-rows of the lo*lo partial correction in pass A
AP_ON_GPSIMD = False  # Q7 tensor_scalar measured ~7.4us per [128,512] op --
                      # far below line rate; keep A' = relu(g-1) on the DVE

_bf16 = ml_dtypes.bfloat16


def _chunks_for(klen):
    return (int(klen) + 127) // 128


def _slot_order(CS):
    """(hh, g) slots in schedule order: larger head first, so the slots
    left without phase-1 overlap at the end are the cheap ones."""
    heads = sorted(range(HEADS_PER_CORE), key=lambda hh: -CS[hh])
    return [(hh, g) for hh in heads for g in range(QB)]


def _ind_groups(CS):
    """Slots run in indicator mode (no Sign; count = sum of 1{d'>0}):
    the final two slots, whose phase 2 has no phase-1 stream left to
    overlap and would otherwise pace on the ScalarE exp+sign pair."""
    return set(_slot_order(CS)[-2:])


def _build_bass(CS):
    """CS: tuple of per-head chunk counts, e.g. (10, 12)."""
    import concourse.mybir as mybir
    from concourse import bacc
    from concourse.tile import TileContext
    from concourse.masks import make_identity
    from collections import deque

    f32 = mybir.dt.float32
    bf16 = mybir.dt.bfloat16

    nc = bacc.Bacc()
    HPC = HEADS_PER_CORE
    assert len(CS) == HPC
    SH = [c * 128 for c in CS]          # padded key count per head
    NS = [(c + 3) // 4 for c in CS]     # 512-wide pf subtiles per L-tile

    qa_d, ka_d, qbc_d, kbc_d, va_d, out_d, nsel_d = [], [], [], [], [], [], []
    for hh in range(HPC):
        qa_d.append(nc.declare_dram_parameter(f"qa{hh}", [128, L], bf16,
                                              isOutput=False))
        ka_d.append(nc.declare_dram_parameter(f"ka{hh}", [128, SH[hh]], bf16,
                                              isOutput=False))
        qbc_d.append(nc.declare_dram_parameter(f"qbc{hh}", [128, L], bf16,
                                               isOutput=False))
        kbc_d.append(nc.declare_dram_parameter(f"kbc{hh}", [128, SH[hh]],
                                               bf16, isOutput=False))
        va_d.append(nc.declare_dram_parameter(f"va{hh}", [CS[hh], 128, D + 1],
                                              bf16, isOutput=False))
        out_d.append(nc.declare_dram_parameter(f"out{hh}", [L, D], f32,
                                               isOutput=True))
        nsel_d.append(nc.declare_dram_parameter(f"nsel{hh}", [L], f32,
                                                isOutput=True))

    from contextlib import ExitStack
    with TileContext(nc) as tc, ExitStack() as ctx:
        consts = ctx.enter_context(tc.tile_pool(name="consts", bufs=1))
        inpool = ctx.enter_context(tc.tile_pool(name="inputs", bufs=1))
        cpool = ctx.enter_context(tc.tile_pool(name="cands", bufs=4))
        small = ctx.enter_context(tc.tile_pool(name="small", bufs=3))
        gs_pool = ctx.enter_context(tc.tile_pool(name="gs", bufs=4))
        opool = ctx.enter_context(tc.tile_pool(name="outbuf", bufs=3))
        ps_f = ctx.enter_context(tc.tile_pool(name="ps_fwd", bufs=2, space="PSUM"))
        ps_t = ctx.enter_context(tc.tile_pool(name="ps_t", bufs=2, space="PSUM"))
        ps_av = ctx.enter_context(tc.tile_pool(name="ps_av", bufs=1, space="PSUM"))
        ps_x = ctx.enter_context(tc.tile_pool(name="ps_x", bufs=1, space="PSUM"))

        ident = consts.tile([128, 128], bf16)
        make_identity(nc, ident)
        ident32 = consts.tile([128, 128], f32)
        make_identity(nc, ident32)
        ones_col = consts.tile([128, 1], bf16)
        nc.vector.memset(ones_col, 1.0)

        # ---- load all inputs; head-0 p1 operands first (they gate the
        # prologue), spread across queues so dispatch doesn't serialize ----
        qa = [None] * HPC
        ka = [None] * HPC
        qbc = [None] * HPC
        kbc = [None] * HPC
        va = [None] * HPC
        for hh in range(HPC):
            qa[hh] = inpool.tile([128, L], bf16, tag=f"qa{hh}", name=f"qa{hh}")
            ka[hh] = inpool.tile([128, SH[hh]], bf16, tag=f"ka{hh}",
                                 name=f"ka{hh}")
            qbc[hh] = inpool.tile([128, L], bf16, tag=f"qbc{hh}",
                                  name=f"qbc{hh}")
            kbc[hh] = inpool.tile([128, SH[hh]], bf16, tag=f"kbc{hh}",
                                  name=f"kbc{hh}")
            va[hh] = inpool.tile([128, CS[hh], D + 1], bf16, tag=f"va{hh}",
                                 name=f"va{hh}")
        slot_order = _slot_order(CS)
        ind_groups = _ind_groups(CS)
        h1 = slot_order[0][0]      # head scheduled first (prologue head)
        h2 = 1 - h1
        # first pieces cover the prologue's operands so phase 1 starts
        # within a couple of microseconds of kernel start
        nc.sync.dma_start(ka[h1][:, 0:512], ka_d[h1][:, 0:512])
        nc.sync.dma_start(qa[h1][:, 0:512], qa_d[h1][:, 0:512])
        nc.scalar.dma_start(kbc[h1][:, 0:512], kbc_d[h1][:, 0:512])
        nc.scalar.dma_start(qbc[h1][:, 0:512], qbc_d[h1][:, 0:512])
        nc.sync.dma_start(ka[h1][:, 512:SH[h1]], ka_d[h1][:, 512:SH[h1]])
        nc.sync.dma_start(qa[h1][:, 512:L], qa_d[h1][:, 512:L])
        nc.scalar.dma_start(kbc[h1][:, 512:SH[h1]], kbc_d[h1][:, 512:SH[h1]])
        nc.scalar.dma_start(qbc[h1][:, 512:L], qbc_d[h1][:, 512:L])
        nc.sync.dma_start(qa[h2], qa_d[h2][:, :])
        nc.sync.dma_start(ka[h2], ka_d[h2][:, :])
        nc.scalar.dma_start(qbc[h2], qbc_d[h2][:, :])
        nc.scalar.dma_start(kbc[h2], kbc_d[h2][:, :])
        nc.gpsimd.dma_start(va[h1], va_d[h1].rearrange("c p d -> p c d"))
        nc.gpsimd.dma_start(va[h2], va_d[h2].rearrange("c p d -> p c d"))

        halfsum = [None] * HPC

        def head_prep(hh):
            # 0.5 * sum_s V_aug
            pv = ps_x.tile([128, 128], f32, tag="tpose", name="tpose")
            for c in range(CS[hh]):
                nc.tensor.matmul(pv[0:D + 1, 0:1], va[hh][:, c, :], ones_col,
                                 start=(c == 0), stop=(c == CS[hh] - 1))
            halfsum[hh] = small.tile([D + 1, 1], f32, tag=f"halfsum{hh}",
                                     name=f"halfsum{hh}")
            nc.scalar.activation(halfsum[hh], pv[0:D + 1, 0:1],
                                 mybir.ActivationFunctionType.Copy, scale=0.5)

        def p1_pf(hh, lt, q4, pool=None, tag="fwd"):
            """forward scores for up to 512 keys of tile lt."""
            w = min(512, SH[hh] - q4 * 512)
            pf = (pool or ps_f).tile([128, 512], f32, tag=tag, name=tag)
            nc.tensor.matmul(pf[:, 0:w], qa[hh][:, lt * 128:(lt + 1) * 128],
                             ka[hh][:, q4 * 512:q4 * 512 + w],
                             start=True, stop=False)
            nc.tensor.matmul(pf[:, 0:w], qbc[hh][:, lt * 128:(lt + 1) * 128],
                             kbc[hh][:, q4 * 512:q4 * 512 + w],
                             start=False, stop=True)
            return pf

        def p1_max8(hh, pf, q4, cands):
            """top-8 of each 128-wide chunk, straight from PSUM."""
            nch = min(4, CS[hh] - 4 * q4)
            for j in range(nch):
                c0 = (4 * q4 + j) * 8
                nc.vector.max(out=cands[:, c0:c0 + 8],
                              in_=pf[:, j * 128:(j + 1) * 128])

        def p1_extract_a(cands, m_sb):
            """extraction rounds 0-1 (max8, mr, max8, mr)."""
            for r in range(2):
                nc.vector.max(out=m_sb[:, 8 * r:8 * r + 8], in_=cands)
                nc.vector.match_replace(
                    out=cands, in_to_replace=m_sb[:, 8 * r:8 * r + 8],
                    in_values=cands, imm_value=NEG)

        def p1_extract_b(i, cands, m_sb, tcol4):
            """extraction rounds 2-3; t32 -> tcol4 col i."""
            nc.vector.max(out=m_sb[:, 16:24], in_=cands)
            nc.vector.match_replace(
                out=cands, in_to_replace=m_sb[:, 16:24],
                in_values=cands, imm_value=NEG)
            nc.vector.max(out=m_sb[:, 24:32], in_=cands)
            nc.vector.tensor_copy(tcol4[:, i:i + 1], m_sb[:, 31:32])

        def p1_split(tcols, tcol4):
            """batched t_minus + bf16 triple split for the 4 tiles.

            m = -(t - |t|*2^-19 - 1e-37) = |t|*2^-19 + 1e-37 - t
            (2^-19, not 1 ulp: phase 2 folds -t into the accumulation
            before the lo-product rows, so its rounding path differs
            from phase 1's by ~±8e-6; the cut needs to clear that.)"""
            acol = small.tile([128, 12], f32, tag="tm", name="tm")
            nc.scalar.activation(acol[:, 0:4], tcol4,
                                 mybir.ActivationFunctionType.Abs,
                                 scale=float(2.0 ** -19))
            nc.vector.scalar_tensor_tensor(
                out=acol[:, 4:8], in0=acol[:, 0:4], scalar=1e-37, in1=tcol4,
                op0=mybir.AluOpType.add, op1=mybir.AluOpType.subtract)
            nc.vector.tensor_copy(tcols[:, 0:4], acol[:, 4:8])
            nc.vector.tensor_tensor(
                out=acol[:, 8:12], in0=acol[:, 4:8], in1=tcols[:, 0:4],
                op=mybir.AluOpType.subtract)
            nc.vector.tensor_copy(tcols[:, 4:8], acol[:, 8:12])
            nc.vector.tensor_tensor(
                out=acol[:, 0:4], in0=acol[:, 8:12], in1=tcols[:, 4:8],
                op=mybir.AluOpType.subtract)
            nc.vector.tensor_copy(tcols[:, 8:12], acol[:, 0:4])

        def p1_stage(hh, g, tcols):
            """transpose tcols into qa rows 65..67, cols of q-group g."""
            pt = ps_x.tile([128, 128], bf16, tag="tposeb", name="tposeb")
            nc.tensor.transpose(pt[0:12, :], tcols, ident)
            stage = small.tile([12, 128], bf16, tag="stage12", name="stage12")
            nc.scalar.copy(out=stage, in_=pt[0:12, :])
            nc.sync.dma_start(
                qa[hh][65:68, g * 512:(g + 1) * 512].rearrange(
                    "p (t q) -> p t q", t=4),
                stage[:, :])

        def p2_pt(hh, g, c):
            qs = slice(g * 512, (g + 1) * 512)
            pt = ps_t.tile([128, 512], f32, tag="psumT", name="psumT")
            nc.tensor.matmul(pt, ka[hh][:, c * 128:(c + 1) * 128],
                             qa[hh][:, qs], start=True, stop=False)
            nc.tensor.matmul(pt, kbc[hh][:, c * 128:(c + 1) * 128],
                             qbc[hh][:, qs], start=False, stop=True)
            return pt

        def p2_act(pt, want_sign):
            g_sb = gs_pool.tile([128, 512], bf16, tag="g", name="g")
            nc.scalar.activation(g_sb, pt,
                                 mybir.ActivationFunctionType.Exp,
                                 scale=float(TEMP))
            if not want_sign:
                return g_sb, None
            s_sb = gs_pool.tile([128, 512], bf16, tag="s", name="s")
            nc.scalar.activation(s_sb, pt,
                                 mybir.ActivationFunctionType.Sign)
            return g_sb, s_sb

        def p2_ind(pt):
            """ind = 1{d' > 0} straight from PSUM (DVE; exact cut -- the
            32nd key sits only ~|t|*2^-19 above t_minus, so the compare
            must happen on the fp32 scores, not on bf16 g)."""
            ind_sb = gs_pool.tile([128, 512], bf16, tag="s", name="ind")
            nc.vector.tensor_scalar(
                out=ind_sb, in0=pt, scalar1=0.0, scalar2=None,
                op0=mybir.AluOpType.is_gt)
            return ind_sb

        def p2_ap(g_sb):
            ap_sb = gs_pool.tile([128, 512], bf16, tag="ap", name="ap")
            eng = nc.gpsimd if AP_ON_GPSIMD else nc.vector
            eng.tensor_scalar(
                out=ap_sb, in0=g_sb, scalar1=1.0, scalar2=0.0,
                op0=mybir.AluOpType.subtract, op1=mybir.AluOpType.max)
            return ap_sb

        def p2_av(hh, c, av_g, av_s, ap_sb, s_sb):
            nc.tensor.matmul(av_g, va[hh][:, c, :], ap_sb,
                             start=(c == 0), stop=(c == CS[hh] - 1))
            nc.tensor.matmul(av_s, va[hh][:, c, :], s_sb,
                             start=(c == 0), stop=(c == CS[hh] - 1))

        def p2_tail_u(hh, g, av_g, av_s):
            # selection count -> host.  Sign mode: row D of av_s is
            # 2*cnt - SH; indicator mode: row D of av_ind is cnt.
            is_ind = (hh, g) in ind_groups
            nsel_sb = opool.tile([1, 512], f32, tag="nsel", name="nsel")
            nc.scalar.copy(out=nsel_sb, in_=av_s[D:D + 1, :])
            nc.scalar.dma_start(nsel_d[hh][g * 512:(g + 1) * 512], nsel_sb)
            # sign mode:      u = (0.5*av_s + halfsum) + av_g
            # indicator mode: u = (1.0*av_ind + 0)     + av_g
            u1_sb = opool.tile([D + 1, 512], f32, tag="u1", name="u1")
            nc.scalar.activation(u1_sb, av_s[0:D + 1, :],
                                 mybir.ActivationFunctionType.Identity,
                                 bias=0.0 if is_ind else halfsum[hh],
                                 scale=1.0 if is_ind else 0.5)
            u_sb = opool.tile([D + 1, 512], f32, tag="u", name="u")
            nc.vector.tensor_tensor(out=u_sb, in0=u1_sb, in1=av_g,
                                    op=mybir.AluOpType.add)
            return u_sb

        def p2_tail_out(hh, g, u_sb, sub):
            po = ps_x.tile([128, 128], f32, tag="tpose", name="tpose")
            nc.tensor.transpose(po[:, 0:D + 1],
                                u_sb[:, sub * 128:(sub + 1) * 128],
                                ident32[0:D + 1, 0:D + 1])
            recip = opool.tile([128, 1], f32, tag="recip", name="recip")
            nc.vector.reciprocal(out=recip, in_=po[:, D:D + 1])
            o_sb = opool.tile([128, D], f32, tag="osb", name="osb")
            nc.vector.tensor_scalar(
                out=o_sb, in0=po[:, 0:D], scalar1=recip, scalar2=None,
                op0=mybir.AluOpType.mult)
            lq = g * 512 + sub * 128
            nc.scalar.dma_start(out_d[hh][lq:lq + 128, :], o_sb)

        # ---- pipeline ----
        # slots = (head, q-group); slot s's phase 2 streams its C chunks
        # while the free-running phase-1 stream (1 subtile per step) works
        # ~1.5 slots ahead, so cut values are staged early.
        slots = slot_order

        # global p1 stream state
        p1q = deque()            # (sidx, hh, g, tile_i, q4)
        for sidx in range(1, len(slots)):
            hh, g = slots[sidx]
            for i in range(4):
                for q4 in range(NS[hh]):
                    p1q.append((sidx, hh, g, i, q4))
        sstate = {}              # sidx -> dict(cands, msb, tcol4, tcols, nx)
        ext_q = deque()          # ('a'|'b', sidx, i)
        pending_pf = None        # (sidx, hh, i, q4, pf) awaiting max8

        def p1_state(sidx, hh, g):
            st = sstate.get(sidx)
            if st is None:
                st = dict(cands={}, msb={}, extracted=0,
                          tcol4=small.tile([128, 4], f32, tag="t4",
                                           name="t4"))
                sstate[sidx] = st
            return st

        def p1_step_issue():
            """issue next pf of the global stream (PE)."""
            nonlocal pending_pf
            if not p1q:
                return
            sidx, hh, g, i, q4 = p1q.popleft()
            st = p1_state(sidx, hh, g)
            if q4 == 0:
                st['cands'][i] = cpool.tile([128, 8 * CS[hh]], f32,
                                            tag=f"cands{hh}", name="cands")
            pf = p1_pf(hh, 4 * g + i, q4)
            pending_pf = (sidx, hh, g, i, q4, pf)

        def p1_step_max8():
            """emit max8s for the pf issued last step (DVE)."""
            nonlocal pending_pf
            if pending_pf is None:
                return
            sidx, hh, g, i, q4, pf = pending_pf
            pending_pf = None
            st = sstate[sidx]
            p1_max8(hh, pf, q4, st['cands'][i])
            if q4 == NS[hh] - 1:
                ext_q.append(('a', sidx, hh, g, i))
                ext_q.append(('b', sidx, hh, g, i))

        def p1_step_extract(budget=2):
            """drain up to `budget` extraction stages (DVE)."""
            while budget > 0 and ext_q:
                kind, sidx, hh, g, i = ext_q.popleft()
                st = sstate[sidx]
                if kind == 'a':
                    st['msb'][i] = small.tile([128, 32], f32, tag="m32",
                                              name="m32")
                    p1_extract_a(st['cands'][i], st['msb'][i])
                else:
                    p1_extract_b(i, st['cands'][i], st['msb'][i],
                                 st['tcol4'])
                    del st['cands'][i]
                    del st['msb'][i]
                    st['extracted'] += 1
                    if st['extracted'] == 4:
                        tcols = small.tile([128, 12], bf16, tag="tcols",
                                           name="tcols")
                        p1_split(tcols, st['tcol4'])
                        p1_stage(hh, g, tcols)
                        del sstate[sidx]
                budget -= 1

        def p1_whole(hh, g, tcols0):
            """prologue phase 1 for slot 0: pf ring borrows the idle ps_t
            bank pair (4-deep pipeline) and max8s trail their pf by one
            subtile so the PE never waits on the DVE."""
            ns = NS[hh]
            nsub = 4 * ns
            tcol4 = small.tile([128, 4], f32, tag="t4", name="t4")
            candss = {}
            msbs = {}
            pfs = {}
            for sc in range(nsub + 3):
                if sc < nsub:
                    i, q4 = sc // ns, sc % ns
                    if q4 == 0:
                        candss[i] = cpool.tile([128, 8 * CS[hh]], f32,
                                               tag=f"cands{hh}",
                                               name="cands")
                    pool, tag = ((ps_f, "fwd") if sc % 2 == 0
                                 else (ps_t, "psumT"))
                    pfs[sc] = p1_pf(hh, 4 * g + i, q4, pool, tag)
                s1 = sc - 1
                if 0 <= s1 < nsub:
                    p1_max8(hh, pfs[s1], s1 % ns, candss[s1 // ns])
                    del pfs[s1]
                s2 = sc - ns - 1
                if s2 >= 0 and s2 % ns == 0 and s2 // ns < 4:
                    i = s2 // ns
                    msbs[i] = small.tile([128, 32], f32, tag="m32",
                                         name="m32")
                    p1_extract_a(candss[i], msbs[i])
                s3 = sc - ns - 2
                if s3 >= 0 and s3 % ns == 0 and s3 // ns < 4:
                    i = s3 // ns
                    p1_extract_b(i, candss[i], msbs[i], tcol4)
            p1_split(tcols0, tcol4)
            p1_stage(hh, g, tcols0)

        # slot 0's cut values are computed on the host (same bf16-split
        # scoring, wider 2^-17 margin) and shipped pre-staged inside qa
        # rows 65..67, so phase 2 starts immediately -- no prologue.
        # head_preps are emitted a few steps into slot 0 so their wait on
        # the va DMAs can't head-block the first pt matmuls.

        prev_tail = None   # (hh, g, av_g, av_s) of the previous slot
        prev_u = None
        for s, (hh, g) in enumerate(slots):
            C = CS[hh]
            is_ind = (hh, g) in ind_groups
            av_g = ps_av.tile([D + 1, 512], f32, tag="av_g", name="av_g")
            av_s = ps_av.tile([D + 1, 512], f32, tag="av_s", name="av_s")
            pts = {}
            gss = {}
            aps = {}
            for step in range(C + 3):
                c, c1, c2 = step, step - 1, step - 2
                # DVE: ap for chunk c2 first (exp finished a step ago), so
                # the av matmuls emitted below stall minimally
                if 0 <= c2 < C:
                    aps[c2] = p2_ap(gss[c2][0])
                # PE: pt(c), then av(c2), then the p1-stream pf
                if c < C:
                    pts[c] = p2_pt(hh, g, c)
                if step == 1 and s == 0:
                    head_prep(hh)           # va[hh] just landed
                if step == 1 and s == 1:
                    head_prep(1 - slots[0][0])   # other head, well after

                if 0 <= c2 < C:
                    p2_av(hh, c2, av_g, av_s, aps[c2], gss[c2][1])
                    del aps[c2]
                p1_step_issue()
                # previous slot's tail, spread over this slot's first steps
                if prev_tail is not None:
                    ph, pg, pav_g, pav_s = prev_tail
                    if step == 0:
                        prev_u = p2_tail_u(ph, pg, pav_g, pav_s)
                    elif step <= 4:
                        p2_tail_out(ph, pg, prev_u, step - 1)
                        if step == 4:
                            prev_tail = None
                # DVE: chunk-max8s of the pf issued last step, then up to
                # two extraction stages
                p1_step_max8()
                p1_step_extract(2)
                # ScalarE: exp (+ sign unless indicator mode) of pt(c1);
                # indicator mode takes 1{d'>0} from PSUM on the DVE while
                # pt(c1) is still live
                if 0 <= c1 < C:
                    g_sb, s_sb = p2_act(pts[c1], want_sign=not is_ind)
                    if is_ind:
                        s_sb = p2_ind(pts[c1])
                    gss[c1] = (g_sb, s_sb)
                    del pts[c1]
            prev_tail = (hh, g, av_g, av_s)
        # drain any remaining p1 work (shouldn't happen) and final tail
        while p1q or ext_q or pending_pf is not None:
            p1_step_issue()
            p1_step_max8()
            p1_step_extract(4)
        ph, pg, pav_g, pav_s = prev_tail
        u_last = p2_tail_u(ph, pg, pav_g, pav_s)
        for sub in range(4):
            p2_tail_out(ph, pg, u_last, sub)

    nc.compile()
    return nc


_NC_CACHE = {}


def _get_nc(CS):
    key = tuple(CS)
    if key not in _NC_CACHE:
        _NC_CACHE[key] = _build_bass(key)
    return _NC_CACHE[key]


def _split_hi_lo(x):
    hi = x.astype(_bf16)
    lo = (x.astype(np.float32) - hi.astype(np.float32)).astype(_bf16)
    return hi, lo


def _host_fix_rows(out, fix_rows, queries, keys, values, key_lengths):
    """Vectorized exact fp32 recompute of rows whose selection count != 32."""
    by_nh = {}
    for (n, lq, h) in fix_rows:
        by_nh.setdefault((n, h), []).append(lq)
    for (n, h), lqs in by_nh.items():
        lqs = np.asarray(sorted(lqs))
        kl = int(key_lengths[n])
        Q = np.asarray(queries[n, lqs, h, :], np.float32)      # [m, E]
        K = np.asarray(keys[n, :kl, h, :], np.float32)         # [kl, E]
        V = np.asarray(values[n, :kl, h, :], np.float32)       # [kl, D]
        s = Q @ K.T                                            # [m, kl]
        idx = np.argpartition(-s, TOPK - 1, axis=1)[:, :TOPK]
        sv = np.take_along_axis(s, idx, axis=1)
        w = np.exp(TEMP * (sv - sv.max(axis=1, keepdims=True)))
        w /= w.sum(axis=1, keepdims=True)
        out[n, lqs, h, :] = np.einsum('mk,mkd->md', w, V[idx])


def _host_t_stage(qh, ql, kh, kl_, mask, nq):
    """Cut values t_minus for the first nq queries, replicating the
    device's bf16-split scoring (pass A + pass BC products in fp32), with
    a 2^-17 margin to absorb fp32 summation-order differences vs PSUM.
    Returns the 3-term bf16 split of m = -t_minus, shape [3, nq]."""
    q1 = qh[:nq].astype(np.float32)
    q2 = ql[:nq].astype(np.float32)
    k1 = kh.astype(np.float32)
    k2 = kl_.astype(np.float32)
    s = q1 @ k1.T + q1 @ k2.T + q2 @ k1.T \
        + q2[:, :NLO] @ k2[:, :NLO].T + mask[None, :]
    t = -np.partition(-s, TOPK - 1, axis=1)[:, TOPK - 1]
    m = (np.abs(t) * (2.0 ** -17) + 1e-37 - t).astype(np.float32)
    m1 = m.astype(_bf16)
    m2 = (m - m1.astype(np.float32)).astype(_bf16)
    m3 = (m - m1.astype(np.float32) - m2.astype(np.float32)).astype(_bf16)
    return np.stack([m1, m2, m3])


def _key_perm(n, klen):
    """Fixed random permutation of the klen valid key slots of batch n.

    The output is invariant to key order, but the candidate-chunk
    prefilter assumes the top-32 positions of a row are spread across
    chunks; scattering valid keys over the compacted range gives every
    chunk ~klen/C valid keys and the balls-in-bins behaviour (rows where
    a chunk overflows are caught by the count check and host-fixed)."""
    rng = np.random.default_rng(1234567 + n)
    return rng.permutation(klen)


def _prep_core(core, queries, keys, values, key_lengths_i):
    pairs = [(0, core), (1, core)]
    CS = [_chunks_for(key_lengths_i[n]) for n, _ in pairs]
    im = {}
    for i, (n, h) in enumerate(pairs):
        klen = int(key_lengths_i[n])
        C = CS[i]
        Sh = C * 128
        perm = _key_perm(n, klen)
        qa = np.zeros((128, L), _bf16)
        ka = np.zeros((128, Sh), _bf16)
        qbc = np.zeros((128, L), _bf16)
        kbc = np.zeros((128, Sh), _bf16)
        va = np.zeros((C, 128, D + 1), _bf16)
        Q = queries[n, :, h, :]                      # [L, E]
        K = np.zeros((Sh, E), np.float32)
        V = np.zeros((Sh, D), np.float32)
        K[:klen] = keys[n, :klen, h, :][perm]
        V[:klen] = values[n, :klen, h, :][perm]
        qh, ql = _split_hi_lo(Q)
        kh, kl_ = _split_hi_lo(K)
        mask = np.full(Sh, NEG, np.float32)
        mask[:klen] = 0.0
        qa[0:E, :] = qh.T
        qa[E, :] = 1.0
        # rows 65..67: t slots, filled on device -- except the first
        # scheduled slot's q-block, whose cut values are host-computed
        qa[E + 4:E + 4 + NLO, :] = ql.T[0:NLO]
        if (i, 0) == _slot_order(CS)[0]:
            qa[65:68, 0:512] = _host_t_stage(qh, ql, kh, kl_, mask, 512)
        ka[0:E, :] = kh.T
        ka[E, :] = mask.astype(_bf16)
        ka[E + 1:E + 4, :] = 1.0
        ka[E + 4:E + 4 + NLO, :] = kl_.T[0:NLO]
        qbc[0:E, :] = qh.T
        qbc[E:2 * E, :] = ql.T
        kbc[0:E, :] = kl_.T
        kbc[E:2 * E, :] = kh.T
        va[:, :, 0:D] = V.astype(_bf16).reshape(C, 128, D)
        va[:, :, D] = 1.0
        im.update({f"qa{i}": qa, f"ka{i}": ka, f"qbc{i}": qbc,
                   f"kbc{i}": kbc, f"va{i}": va})
    return pairs, CS, im


def kernel(queries, keys, values, key_lengths):
    from concourse.bass_utils import run_bass_kernel_spmd

    queries = np.asarray(queries, np.float32)
    keys = np.asarray(keys, np.float32)
    values = np.asarray(values, np.float32)
    key_lengths_i = np.asarray(key_lengths).astype(np.int64)

    in_maps = []
    head_map = []  # per core: list of (n, h)
    CS = None
    for core in range(N_CORES):
        pairs, CS, im = _prep_core(core, queries, keys, values,
                                   key_lengths_i)
        head_map.append(pairs)
        in_maps.append(im)

    nc = _get_nc(CS)
    res = run_bass_kernel_spmd(nc, in_maps, list(range(N_CORES)))

    out = np.zeros((N, L, H, D), np.float32)
    ind_groups = _ind_groups(CS)
    fix_rows = []
    for core in range(N_CORES):
        for i, (n, h) in enumerate(head_map[core]):
            o = res.results[core][f"out{i}"].reshape(L, D)
            nsel = res.results[core][f"nsel{i}"].reshape(L)
            out[n, :, h, :] = o
            cnt = (nsel + CS[i] * 128) * 0.5
            for g in range(QB):
                if (i, g) in ind_groups:
                    cnt[g * 512:(g + 1) * 512] = nsel[g * 512:(g + 1) * 512]
            bad = np.nonzero(cnt != TOPK)[0]
            for lq in bad:
                fix_rows.append((n, int(lq), h))
    if fix_rows:
        _host_fix_rows(out, fix_rows, queries, keys, values, key_lengths_i)
    return out


# revision 33
# speedup vs baseline: 1.0755x; 1.0047x over previous
"""Exact top-k (k=32) attention on 8 Trainium2 NeuronCores.

Strategy (head-parallel + key compaction): the 16 (batch, head) pairs are
sharded 2-per-core; core i gets (n=0, h=i) and (n=1, h=i), so every core
holds one head of each batch item and the per-core work is identical even
though the two batch items have different key_lengths.

Key compaction: keys beyond key_lengths[n] can never be selected (the
reference masks them to -inf), so only the first klen_n keys are shipped,
permuted, and padded up to C_n = ceil(klen_n/128) chunks of 128.  All
s-proportional work (score matmuls, selection scans, exp/sign, AV) runs
over C_n chunks instead of S/128 = 16.  The bass program is built on the
first kernel() call from the actual key_lengths (cached per (C0, C1)).

Per head, per core:
  Phase 1 (selection): forward scores F[q, s] via a 2-pass bf16-split matmul
    (hi*hi + partial lo*lo folded into pass A; hi*lo + lo*hi in pass BC;
    accurate to ~1e-5); candidate top-8 of each 128-wide key chunk read
    straight from PSUM by C narrow DVE max8 ops, then the 32nd-largest of
    the 8C candidates via 4 rounds of max8 / match_replace.  This equals
    the row's exact 32nd-largest unless one chunk holds >= 9 of the row's
    top-32; such rows make the on-device selection count exceed 32 and are
    recomputed on the host (vectorized) like tie rows.  Cut value
    t_minus = t - |t|*2^-19 - 1e-37, strictly inside (s_33, s_32].
  Phase 2 (apply): transposed scores minus t_minus computed directly by an
    augmented matmul (extra contraction rows: klen mask x ones, ones x
    (-t1,-t2,-t3) with t decomposed into 3 bf16 terms), giving
    d'[s, q] = scores^T - t_minus in PSUM (bit-identical products to the
    forward pass).  Then
      g = Exp(temp*d')        (ScalarE, bf16)
      S = Sign(d')            (ScalarE, bf16, in {-1,+1})
      A' = max(g - 1, 0)      (GPSIMD, bf16) == (w - 1) on selected, 0 off
    and AV is reconstructed via
      sum_sel w*V = V^T A' + 0.5*(V^T S + sum_s V)
    using an appended ones-column of V to carry Z = sum_sel w and the
    selection count.
  Phase 1 runs as a free-running stream ~1 subtile per phase-2 chunk,
  decoupled from the slot boundaries, so each slot's cut values are staged
  well before its phase 2 begins and the PE never goes idle at a slot
  boundary (HAM stays warm).
  A per-row selection count is returned; rows where it is not exactly 32
  (candidate-segment overflow, or s_33 within ~2^-19*|t| of s_32) are
  recomputed exactly on the host with a vectorized numpy path.
"""

import numpy as np
import ml_dtypes

N, L, S, H, E, D = 2, 2048, 2048, 8, 64, 64
TOPK = 32
TEMP = 1.0 / np.sqrt(E)
HEADS_PER_CORE = 2
N_CORES = 8
LT = 16          # L tiles of 128
QB = 4           # q blocks of 512 in phase 2
NEG = -1e30
NLO = 60         # e-rows of the lo*lo partial correction in pass A
AP_ON_GPSIMD = False  # Q7 tensor_scalar measured ~7.4us per [128,512] op --
                      # far below line rate; keep A' = relu(g-1) on the DVE

_bf16 = ml_dtypes.bfloat16


def _chunks_for(klen):
    return (int(klen) + 127) // 128


def _slot_order(CS):
    """(hh, g) slots in schedule order: larger head first, so the slots
    left without phase-1 overlap at the end are the cheap ones."""
    heads = sorted(range(HEADS_PER_CORE), key=lambda hh: -CS[hh])
    return [(hh, g) for hh in heads for g in range(QB)]


def _ind_groups(CS):
    """Slots run in indicator mode (no Sign; count = sum of 1{d'>0}):
    the final two slots, whose phase 2 has no phase-1 stream left to
    overlap and would otherwise pace on the ScalarE exp+sign pair."""
    return set(_slot_order(CS)[-2:])


def _build_bass(CS):
    """CS: tuple of per-head chunk counts, e.g. (10, 12)."""
    import concourse.mybir as mybir
    from concourse import bacc
    from concourse.tile import TileContext
    from concourse.masks import make_identity
    from collections import deque

    f32 = mybir.dt.float32
    bf16 = mybir.dt.bfloat16

    nc = bacc.Bacc()
    HPC = HEADS_PER_CORE
    assert len(CS) == HPC
    SH = [c * 128 for c in CS]          # padded key count per head
    NS = [(c + 3) // 4 for c in CS]     # 512-wide pf subtiles per L-tile

    qa_d, ka_d, qbc_d, kbc_d, va_d, out_d, nsel_d = [], [], [], [], [], [], []
    for hh in range(HPC):
        qa_d.append(nc.declare_dram_parameter(f"qa{hh}", [128, L], bf16,
                                              isOutput=False))
        ka_d.append(nc.declare_dram_parameter(f"ka{hh}", [128, SH[hh]], bf16,
                                              isOutput=False))
        qbc_d.append(nc.declare_dram_parameter(f"qbc{hh}", [128, L], bf16,
                                               isOutput=False))
        kbc_d.append(nc.declare_dram_parameter(f"kbc{hh}", [128, SH[hh]],
                                               bf16, isOutput=False))
        va_d.append(nc.declare_dram_parameter(f"va{hh}", [CS[hh], 128, D + 1],
                                              bf16, isOutput=False))
        out_d.append(nc.declare_dram_parameter(f"out{hh}", [L, D], f32,
                                               isOutput=True))
        nsel_d.append(nc.declare_dram_parameter(f"nsel{hh}", [L], f32,
                                                isOutput=True))

    from contextlib import ExitStack
    with TileContext(nc) as tc, ExitStack() as ctx:
        consts = ctx.enter_context(tc.tile_pool(name="consts", bufs=1))
        inpool = ctx.enter_context(tc.tile_pool(name="inputs", bufs=1))
        cpool = ctx.enter_context(tc.tile_pool(name="cands", bufs=4))
        small = ctx.enter_context(tc.tile_pool(name="small", bufs=3))
        gs_pool = ctx.enter_context(tc.tile_pool(name="gs", bufs=4))
        opool = ctx.enter_context(tc.tile_pool(name="outbuf", bufs=3))
        ps_f = ctx.enter_context(tc.tile_pool(name="ps_fwd", bufs=2, space="PSUM"))
        ps_t = ctx.enter_context(tc.tile_pool(name="ps_t", bufs=2, space="PSUM"))
        ps_av = ctx.enter_context(tc.tile_pool(name="ps_av", bufs=1, space="PSUM"))
        ps_x = ctx.enter_context(tc.tile_pool(name="ps_x", bufs=1, space="PSUM"))

        ident = consts.tile([128, 128], bf16)
        make_identity(nc, ident)
        ident32 = consts.tile([128, 128], f32)
        make_identity(nc, ident32)
        ones_col = consts.tile([128, 1], bf16)
        nc.vector.memset(ones_col, 1.0)

        # ---- load all inputs; head-0 p1 operands first (they gate the
        # prologue), spread across queues so dispatch doesn't serialize ----
        qa = [None] * HPC
        ka = [None] * HPC
        qbc = [None] * HPC
        kbc = [None] * HPC
        va = [None] * HPC
        for hh in range(HPC):
            qa[hh] = inpool.tile([128, L], bf16, tag=f"qa{hh}", name=f"qa{hh}")
            ka[hh] = inpool.tile([128, SH[hh]], bf16, tag=f"ka{hh}",
                                 name=f"ka{hh}")
            qbc[hh] = inpool.tile([128, L], bf16, tag=f"qbc{hh}",
                                  name=f"qbc{hh}")
            kbc[hh] = inpool.tile([128, SH[hh]], bf16, tag=f"kbc{hh}",
                                  name=f"kbc{hh}")
            va[hh] = inpool.tile([128, CS[hh], D + 1], bf16, tag=f"va{hh}",
                                 name=f"va{hh}")
        slot_order = _slot_order(CS)
        ind_groups = _ind_groups(CS)
        h1 = slot_order[0][0]      # head scheduled first (prologue head)
        h2 = 1 - h1
        # first pieces cover the prologue's operands so phase 1 starts
        # within a couple of microseconds of kernel start
        # piece 1: slot-0 phase 2 (ka full-ish + qa group 0) AND the p1
        # stream's first tiles (qa cols 512:1024 for slot 1)
        nc.sync.dma_start(ka[h1][:, 0:512], ka_d[h1][:, 0:512])
        nc.sync.dma_start(qa[h1][:, 0:1024], qa_d[h1][:, 0:1024])
        nc.scalar.dma_start(kbc[h1][:, 0:512], kbc_d[h1][:, 0:512])
        nc.scalar.dma_start(qbc[h1][:, 0:1024], qbc_d[h1][:, 0:1024])
        nc.sync.dma_start(ka[h1][:, 512:SH[h1]], ka_d[h1][:, 512:SH[h1]])
        nc.sync.dma_start(qa[h1][:, 1024:L], qa_d[h1][:, 1024:L])
        nc.scalar.dma_start(kbc[h1][:, 512:SH[h1]], kbc_d[h1][:, 512:SH[h1]])
        nc.scalar.dma_start(qbc[h1][:, 1024:L], qbc_d[h1][:, 1024:L])
        nc.sync.dma_start(qa[h2], qa_d[h2][:, :])
        nc.sync.dma_start(ka[h2], ka_d[h2][:, :])
        nc.scalar.dma_start(qbc[h2], qbc_d[h2][:, :])
        nc.scalar.dma_start(kbc[h2], kbc_d[h2][:, :])
        nc.gpsimd.dma_start(va[h1], va_d[h1].rearrange("c p d -> p c d"))
        nc.gpsimd.dma_start(va[h2], va_d[h2].rearrange("c p d -> p c d"))

        halfsum = [None] * HPC

        def head_prep(hh):
            # 0.5 * sum_s V_aug
            pv = ps_x.tile([128, 128], f32, tag="tpose", name="tpose")
            for c in range(CS[hh]):
                nc.tensor.matmul(pv[0:D + 1, 0:1], va[hh][:, c, :], ones_col,
                                 start=(c == 0), stop=(c == CS[hh] - 1))
            halfsum[hh] = small.tile([D + 1, 1], f32, tag=f"halfsum{hh}",
                                     name=f"halfsum{hh}")
            nc.scalar.activation(halfsum[hh], pv[0:D + 1, 0:1],
                                 mybir.ActivationFunctionType.Copy, scale=0.5)

        def p1_pf(hh, lt, q4, pool=None, tag="fwd"):
            """forward scores for up to 512 keys of tile lt."""
            w = min(512, SH[hh] - q4 * 512)
            pf = (pool or ps_f).tile([128, 512], f32, tag=tag, name=tag)
            nc.tensor.matmul(pf[:, 0:w], qa[hh][:, lt * 128:(lt + 1) * 128],
                             ka[hh][:, q4 * 512:q4 * 512 + w],
                             start=True, stop=False)
            nc.tensor.matmul(pf[:, 0:w], qbc[hh][:, lt * 128:(lt + 1) * 128],
                             kbc[hh][:, q4 * 512:q4 * 512 + w],
                             start=False, stop=True)
            return pf

        def p1_max8(hh, pf, q4, cands):
            """top-8 of each 128-wide chunk, straight from PSUM."""
            nch = min(4, CS[hh] - 4 * q4)
            for j in range(nch):
                c0 = (4 * q4 + j) * 8
                nc.vector.max(out=cands[:, c0:c0 + 8],
                              in_=pf[:, j * 128:(j + 1) * 128])

        def p1_extract_a(cands, m_sb):
            """extraction rounds 0-1 (max8, mr, max8, mr)."""
            for r in range(2):
                nc.vector.max(out=m_sb[:, 8 * r:8 * r + 8], in_=cands)
                nc.vector.match_replace(
                    out=cands, in_to_replace=m_sb[:, 8 * r:8 * r + 8],
                    in_values=cands, imm_value=NEG)

        def p1_extract_b(i, cands, m_sb, tcol4):
            """extraction rounds 2-3; t32 -> tcol4 col i."""
            nc.vector.max(out=m_sb[:, 16:24], in_=cands)
            nc.vector.match_replace(
                out=cands, in_to_replace=m_sb[:, 16:24],
                in_values=cands, imm_value=NEG)
            nc.vector.max(out=m_sb[:, 24:32], in_=cands)
            nc.vector.tensor_copy(tcol4[:, i:i + 1], m_sb[:, 31:32])

        def p1_split(tcols, tcol4):
            """batched t_minus + bf16 triple split for the 4 tiles.

            m = -(t - |t|*2^-19 - 1e-37) = |t|*2^-19 + 1e-37 - t
            (2^-19, not 1 ulp: phase 2 folds -t into the accumulation
            before the lo-product rows, so its rounding path differs
            from phase 1's by ~±8e-6; the cut needs to clear that.)"""
            acol = small.tile([128, 12], f32, tag="tm", name="tm")
            nc.scalar.activation(acol[:, 0:4], tcol4,
                                 mybir.ActivationFunctionType.Abs,
                                 scale=float(2.0 ** -19))
            nc.vector.scalar_tensor_tensor(
                out=acol[:, 4:8], in0=acol[:, 0:4], scalar=1e-37, in1=tcol4,
                op0=mybir.AluOpType.add, op1=mybir.AluOpType.subtract)
            nc.vector.tensor_copy(tcols[:, 0:4], acol[:, 4:8])
            nc.vector.tensor_tensor(
                out=acol[:, 8:12], in0=acol[:, 4:8], in1=tcols[:, 0:4],
                op=mybir.AluOpType.subtract)
            nc.vector.tensor_copy(tcols[:, 4:8], acol[:, 8:12])
            nc.vector.tensor_tensor(
                out=acol[:, 0:4], in0=acol[:, 8:12], in1=tcols[:, 4:8],
                op=mybir.AluOpType.subtract)
            nc.vector.tensor_copy(tcols[:, 8:12], acol[:, 0:4])

        def p1_stage(hh, g, tcols):
            """transpose tcols into qa rows 65..67, cols of q-group g."""
            pt = ps_x.tile([128, 128], bf16, tag="tposeb", name="tposeb")
            nc.tensor.transpose(pt[0:12, :], tcols, ident)
            stage = small.tile([12, 128], bf16, tag="stage12", name="stage12")
            nc.scalar.copy(out=stage, in_=pt[0:12, :])
            nc.sync.dma_start(
                qa[hh][65:68, g * 512:(g + 1) * 512].rearrange(
                    "p (t q) -> p t q", t=4),
                stage[:, :])

        def p2_pt(hh, g, c):
            qs = slice(g * 512, (g + 1) * 512)
            pt = ps_t.tile([128, 512], f32, tag="psumT", name="psumT")
            nc.tensor.matmul(pt, ka[hh][:, c * 128:(c + 1) * 128],
                             qa[hh][:, qs], start=True, stop=False)
            nc.tensor.matmul(pt, kbc[hh][:, c * 128:(c + 1) * 128],
                             qbc[hh][:, qs], start=False, stop=True)
            return pt

        def p2_act(pt, want_sign):
            g_sb = gs_pool.tile([128, 512], bf16, tag="g", name="g")
            nc.scalar.activation(g_sb, pt,
                                 mybir.ActivationFunctionType.Exp,
                                 scale=float(TEMP))
            if not want_sign:
                return g_sb, None
            s_sb = gs_pool.tile([128, 512], bf16, tag="s", name="s")
            nc.scalar.activation(s_sb, pt,
                                 mybir.ActivationFunctionType.Sign)
            return g_sb, s_sb

        def p2_ind(pt):
            """ind = 1{d' > 0} straight from PSUM (DVE; exact cut -- the
            32nd key sits only ~|t|*2^-19 above t_minus, so the compare
            must happen on the fp32 scores, not on bf16 g)."""
            ind_sb = gs_pool.tile([128, 512], bf16, tag="s", name="ind")
            nc.vector.tensor_scalar(
                out=ind_sb, in0=pt, scalar1=0.0, scalar2=None,
                op0=mybir.AluOpType.is_gt)
            return ind_sb

        def p2_ap(g_sb):
            ap_sb = gs_pool.tile([128, 512], bf16, tag="ap", name="ap")
            eng = nc.gpsimd if AP_ON_GPSIMD else nc.vector
            eng.tensor_scalar(
                out=ap_sb, in0=g_sb, scalar1=1.0, scalar2=0.0,
                op0=mybir.AluOpType.subtract, op1=mybir.AluOpType.max)
            return ap_sb

        def p2_av(hh, c, av_g, av_s, ap_sb, s_sb):
            nc.tensor.matmul(av_g, va[hh][:, c, :], ap_sb,
                             start=(c == 0), stop=(c == CS[hh] - 1))
            nc.tensor.matmul(av_s, va[hh][:, c, :], s_sb,
                             start=(c == 0), stop=(c == CS[hh] - 1))

        def p2_tail_u(hh, g, av_g, av_s):
            # selection count -> host.  Sign mode: row D of av_s is
            # 2*cnt - SH; indicator mode: row D of av_ind is cnt.
            is_ind = (hh, g) in ind_groups
            nsel_sb = opool.tile([1, 512], f32, tag="nsel", name="nsel")
            nc.scalar.copy(out=nsel_sb, in_=av_s[D:D + 1, :])
            nc.scalar.dma_start(nsel_d[hh][g * 512:(g + 1) * 512], nsel_sb)
            # sign mode:      u = (0.5*av_s + halfsum) + av_g
            # indicator mode: u = (1.0*av_ind + 0)     + av_g
            u1_sb = opool.tile([D + 1, 512], f32, tag="u1", name="u1")
            nc.scalar.activation(u1_sb, av_s[0:D + 1, :],
                                 mybir.ActivationFunctionType.Identity,
                                 bias=0.0 if is_ind else halfsum[hh],
                                 scale=1.0 if is_ind else 0.5)
            u_sb = opool.tile([D + 1, 512], f32, tag="u", name="u")
            nc.vector.tensor_tensor(out=u_sb, in0=u1_sb, in1=av_g,
                                    op=mybir.AluOpType.add)
            return u_sb

        def p2_tail_out(hh, g, u_sb, sub):
            po = ps_x.tile([128, 128], f32, tag="tpose", name="tpose")
            nc.tensor.transpose(po[:, 0:D + 1],
                                u_sb[:, sub * 128:(sub + 1) * 128],
                                ident32[0:D + 1, 0:D + 1])
            recip = opool.tile([128, 1], f32, tag="recip", name="recip")
            nc.vector.reciprocal(out=recip, in_=po[:, D:D + 1])
            o_sb = opool.tile([128, D], f32, tag="osb", name="osb")
            nc.vector.tensor_scalar(
                out=o_sb, in0=po[:, 0:D], scalar1=recip, scalar2=None,
                op0=mybir.AluOpType.mult)
            lq = g * 512 + sub * 128
            nc.scalar.dma_start(out_d[hh][lq:lq + 128, :], o_sb)

        # ---- pipeline ----
        # slots = (head, q-group); slot s's phase 2 streams its C chunks
        # while the free-running phase-1 stream (1 subtile per step) works
        # ~1.5 slots ahead, so cut values are staged early.
        slots = slot_order

        # global p1 stream state
        p1q = deque()            # (sidx, hh, g, tile_i, q4)
        for sidx in range(1, len(slots)):
            hh, g = slots[sidx]
            for i in range(4):
                for q4 in range(NS[hh]):
                    p1q.append((sidx, hh, g, i, q4))
        sstate = {}              # sidx -> dict(cands, msb, tcol4, tcols, nx)
        ext_q = deque()          # ('a'|'b', sidx, i)
        pending_pf = None        # (sidx, hh, i, q4, pf) awaiting max8

        def p1_state(sidx, hh, g):
            st = sstate.get(sidx)
            if st is None:
                st = dict(cands={}, msb={}, extracted=0,
                          tcol4=small.tile([128, 4], f32, tag="t4",
                                           name="t4"))
                sstate[sidx] = st
            return st

        def p1_step_issue():
            """issue next pf of the global stream (PE)."""
            nonlocal pending_pf
            if not p1q:
                return
            sidx, hh, g, i, q4 = p1q.popleft()
            st = p1_state(sidx, hh, g)
            if q4 == 0:
                st['cands'][i] = cpool.tile([128, 8 * CS[hh]], f32,
                                            tag=f"cands{hh}", name="cands")
            pf = p1_pf(hh, 4 * g + i, q4)
            pending_pf = (sidx, hh, g, i, q4, pf)

        def p1_step_max8():
            """emit max8s for the pf issued last step (DVE)."""
            nonlocal pending_pf
            if pending_pf is None:
                return
            sidx, hh, g, i, q4, pf = pending_pf
            pending_pf = None
            st = sstate[sidx]
            p1_max8(hh, pf, q4, st['cands'][i])
            if q4 == NS[hh] - 1:
                ext_q.append(('a', sidx, hh, g, i))
                ext_q.append(('b', sidx, hh, g, i))

        def p1_step_extract(budget=2):
            """drain up to `budget` extraction stages (DVE)."""
            while budget > 0 and ext_q:
                kind, sidx, hh, g, i = ext_q.popleft()
                st = sstate[sidx]
                if kind == 'a':
                    st['msb'][i] = small.tile([128, 32], f32, tag="m32",
                                              name="m32")
                    p1_extract_a(st['cands'][i], st['msb'][i])
                else:
                    p1_extract_b(i, st['cands'][i], st['msb'][i],
                                 st['tcol4'])
                    del st['cands'][i]
                    del st['msb'][i]
                    st['extracted'] += 1
                    if st['extracted'] == 4:
                        tcols = small.tile([128, 12], bf16, tag="tcols",
                                           name="tcols")
                        p1_split(tcols, st['tcol4'])
                        p1_stage(hh, g, tcols)
                        del sstate[sidx]
                budget -= 1

        def p1_whole(hh, g, tcols0):
            """prologue phase 1 for slot 0: pf ring borrows the idle ps_t
            bank pair (4-deep pipeline) and max8s trail their pf by one
            subtile so the PE never waits on the DVE."""
            ns = NS[hh]
            nsub = 4 * ns
            tcol4 = small.tile([128, 4], f32, tag="t4", name="t4")
            candss = {}
            msbs = {}
            pfs = {}
            for sc in range(nsub + 3):
                if sc < nsub:
                    i, q4 = sc // ns, sc % ns
                    if q4 == 0:
                        candss[i] = cpool.tile([128, 8 * CS[hh]], f32,
                                               tag=f"cands{hh}",
                                               name="cands")
                    pool, tag = ((ps_f, "fwd") if sc % 2 == 0
                                 else (ps_t, "psumT"))
                    pfs[sc] = p1_pf(hh, 4 * g + i, q4, pool, tag)
                s1 = sc - 1
                if 0 <= s1 < nsub:
                    p1_max8(hh, pfs[s1], s1 % ns, candss[s1 // ns])
                    del pfs[s1]
                s2 = sc - ns - 1
                if s2 >= 0 and s2 % ns == 0 and s2 // ns < 4:
                    i = s2 // ns
                    msbs[i] = small.tile([128, 32], f32, tag="m32",
                                         name="m32")
                    p1_extract_a(candss[i], msbs[i])
                s3 = sc - ns - 2
                if s3 >= 0 and s3 % ns == 0 and s3 // ns < 4:
                    i = s3 // ns
                    p1_extract_b(i, candss[i], msbs[i], tcol4)
            p1_split(tcols0, tcol4)
            p1_stage(hh, g, tcols0)

        # slot 0's cut values are computed on the host (same bf16-split
        # scoring, wider 2^-17 margin) and shipped pre-staged inside qa
        # rows 65..67, so phase 2 starts immediately -- no prologue.
        # head_preps are emitted a few steps into slot 0 so their wait on
        # the va DMAs can't head-block the first pt matmuls.

        prev_tail = None   # (hh, g, av_g, av_s) of the previous slot
        prev_u = None
        for s, (hh, g) in enumerate(slots):
            C = CS[hh]
            is_ind = (hh, g) in ind_groups
            av_g = ps_av.tile([D + 1, 512], f32, tag="av_g", name="av_g")
            av_s = ps_av.tile([D + 1, 512], f32, tag="av_s", name="av_s")
            pts = {}
            gss = {}
            aps = {}
            for step in range(C + 3):
                c, c1, c2 = step, step - 1, step - 2
                # DVE: ap for chunk c2 first (exp finished a step ago), so
                # the av matmuls emitted below stall minimally
                if 0 <= c2 < C:
                    aps[c2] = p2_ap(gss[c2][0])
                # PE: pt(c), then av(c2), then the p1-stream pf
                if c < C:
                    pts[c] = p2_pt(hh, g, c)
                if step == 1 and s == 0:
                    head_prep(hh)           # va[hh] just landed
                if step == 1 and s == 1:
                    head_prep(1 - slots[0][0])   # other head, well after

                if 0 <= c2 < C:
                    p2_av(hh, c2, av_g, av_s, aps[c2], gss[c2][1])
                    del aps[c2]
                p1_step_issue()
                # previous slot's tail, spread over this slot's first steps
                if prev_tail is not None:
                    ph, pg, pav_g, pav_s = prev_tail
                    if step == 0:
                        prev_u = p2_tail_u(ph, pg, pav_g, pav_s)
                    elif step <= 4:
                        p2_tail_out(ph, pg, prev_u, step - 1)
                        if step == 4:
                            prev_tail = None
                # DVE: chunk-max8s of the pf issued last step, then up to
                # two extraction stages
                p1_step_max8()
                p1_step_extract(2)
                # ScalarE: exp (+ sign unless indicator mode) of pt(c1);
                # indicator mode takes 1{d'>0} from PSUM on the DVE while
                # pt(c1) is still live
                if 0 <= c1 < C:
                    g_sb, s_sb = p2_act(pts[c1], want_sign=not is_ind)
                    if is_ind:
                        s_sb = p2_ind(pts[c1])
                    gss[c1] = (g_sb, s_sb)
                    del pts[c1]
            prev_tail = (hh, g, av_g, av_s)
        # drain any remaining p1 work (shouldn't happen) and final tail
        while p1q or ext_q or pending_pf is not None:
            p1_step_issue()
            p1_step_max8()
            p1_step_extract(4)
        ph, pg, pav_g, pav_s = prev_tail
        u_last = p2_tail_u(ph, pg, pav_g, pav_s)
        for sub in range(4):
            p2_tail_out(ph, pg, u_last, sub)

    nc.compile()
    return nc


_NC_CACHE = {}


def _get_nc(CS):
    key = tuple(CS)
    if key not in _NC_CACHE:
        _NC_CACHE[key] = _build_bass(key)
    return _NC_CACHE[key]


def _split_hi_lo(x):
    hi = x.astype(_bf16)
    lo = (x.astype(np.float32) - hi.astype(np.float32)).astype(_bf16)
    return hi, lo


def _host_fix_rows(out, fix_rows, queries, keys, values, key_lengths):
    """Vectorized exact fp32 recompute of rows whose selection count != 32."""
    by_nh = {}
    for (n, lq, h) in fix_rows:
        by_nh.setdefault((n, h), []).append(lq)
    for (n, h), lqs in by_nh.items():
        lqs = np.asarray(sorted(lqs))
        kl = int(key_lengths[n])
        Q = np.asarray(queries[n, lqs, h, :], np.float32)      # [m, E]
        K = np.asarray(keys[n, :kl, h, :], np.float32)         # [kl, E]
        V = np.asarray(values[n, :kl, h, :], np.float32)       # [kl, D]
        s = Q @ K.T                                            # [m, kl]
        idx = np.argpartition(-s, TOPK - 1, axis=1)[:, :TOPK]
        sv = np.take_along_axis(s, idx, axis=1)
        w = np.exp(TEMP * (sv - sv.max(axis=1, keepdims=True)))
        w /= w.sum(axis=1, keepdims=True)
        out[n, lqs, h, :] = np.einsum('mk,mkd->md', w, V[idx])


def _host_t_stage(qh, ql, kh, kl_, mask, nq):
    """Cut values t_minus for the first nq queries, replicating the
    device's bf16-split scoring (pass A + pass BC products in fp32), with
    a 2^-17 margin to absorb fp32 summation-order differences vs PSUM.
    Returns the 3-term bf16 split of m = -t_minus, shape [3, nq]."""
    q1 = qh[:nq].astype(np.float32)
    q2 = ql[:nq].astype(np.float32)
    k1 = kh.astype(np.float32)
    k2 = kl_.astype(np.float32)
    s = q1 @ k1.T + q1 @ k2.T + q2 @ k1.T \
        + q2[:, :NLO] @ k2[:, :NLO].T + mask[None, :]
    t = -np.partition(-s, TOPK - 1, axis=1)[:, TOPK - 1]
    m = (np.abs(t) * (2.0 ** -17) + 1e-37 - t).astype(np.float32)
    m1 = m.astype(_bf16)
    m2 = (m - m1.astype(np.float32)).astype(_bf16)
    m3 = (m - m1.astype(np.float32) - m2.astype(np.float32)).astype(_bf16)
    return np.stack([m1, m2, m3])


def _key_perm(n, klen):
    """Fixed random permutation of the klen valid key slots of batch n.

    The output is invariant to key order, but the candidate-chunk
    prefilter assumes the top-32 positions of a row are spread across
    chunks; scattering valid keys over the compacted range gives every
    chunk ~klen/C valid keys and the balls-in-bins behaviour (rows where
    a chunk overflows are caught by the count check and host-fixed)."""
    rng = np.random.default_rng(1234567 + n)
    return rng.permutation(klen)


def _prep_core(core, queries, keys, values, key_lengths_i):
    pairs = [(0, core), (1, core)]
    CS = [_chunks_for(key_lengths_i[n]) for n, _ in pairs]
    im = {}
    for i, (n, h) in enumerate(pairs):
        klen = int(key_lengths_i[n])
        C = CS[i]
        Sh = C * 128
        perm = _key_perm(n, klen)
        qa = np.zeros((128, L), _bf16)
        ka = np.zeros((128, Sh), _bf16)
        qbc = np.zeros((128, L), _bf16)
        kbc = np.zeros((128, Sh), _bf16)
        va = np.zeros((C, 128, D + 1), _bf16)
        Q = queries[n, :, h, :]                      # [L, E]
        K = np.zeros((Sh, E), np.float32)
        V = np.zeros((Sh, D), np.float32)
        K[:klen] = keys[n, :klen, h, :][perm]
        V[:klen] = values[n, :klen, h, :][perm]
        qh, ql = _split_hi_lo(Q)
        kh, kl_ = _split_hi_lo(K)
        mask = np.full(Sh, NEG, np.float32)
        mask[:klen] = 0.0
        qa[0:E, :] = qh.T
        qa[E, :] = 1.0
        # rows 65..67: t slots, filled on device -- except the first
        # scheduled slot's q-block, whose cut values are host-computed
        qa[E + 4:E + 4 + NLO, :] = ql.T[0:NLO]
        if (i, 0) == _slot_order(CS)[0]:
            qa[65:68, 0:512] = _host_t_stage(qh, ql, kh, kl_, mask, 512)
        ka[0:E, :] = kh.T
        ka[E, :] = mask.astype(_bf16)
        ka[E + 1:E + 4, :] = 1.0
        ka[E + 4:E + 4 + NLO, :] = kl_.T[0:NLO]
        qbc[0:E, :] = qh.T
        qbc[E:2 * E, :] = ql.T
        kbc[0:E, :] = kl_.T
        kbc[E:2 * E, :] = kh.T
        va[:, :, 0:D] = V.astype(_bf16).reshape(C, 128, D)
        va[:, :, D] = 1.0
        im.update({f"qa{i}": qa, f"ka{i}": ka, f"qbc{i}": qbc,
                   f"kbc{i}": kbc, f"va{i}": va})
    return pairs, CS, im


def kernel(queries, keys, values, key_lengths):
    from concourse.bass_utils import run_bass_kernel_spmd

    queries = np.asarray(queries, np.float32)
    keys = np.asarray(keys, np.float32)
    values = np.asarray(values, np.float32)
    key_lengths_i = np.asarray(key_lengths).astype(np.int64)

    in_maps = []
    head_map = []  # per core: list of (n, h)
    CS = None
    for core in range(N_CORES):
        pairs, CS, im = _prep_core(core, queries, keys, values,
                                   key_lengths_i)
        head_map.append(pairs)
        in_maps.append(im)

    nc = _get_nc(CS)
    res = run_bass_kernel_spmd(nc, in_maps, list(range(N_CORES)))

    out = np.zeros((N, L, H, D), np.float32)
    ind_groups = _ind_groups(CS)
    fix_rows = []
    for core in range(N_CORES):
        for i, (n, h) in enumerate(head_map[core]):
            o = res.results[core][f"out{i}"].reshape(L, D)
            nsel = res.results[core][f"nsel{i}"].reshape(L)
            out[n, :, h, :] = o
            cnt = (nsel + CS[i] * 128) * 0.5
            for g in range(QB):
                if (i, g) in ind_groups:
                    cnt[g * 512:(g + 1) * 512] = nsel[g * 512:(g + 1) * 512]
            bad = np.nonzero(cnt != TOPK)[0]
            for lq in bad:
                fix_rows.append((n, int(lq), h))
    if fix_rows:
        _host_fix_rows(out, fix_rows, queries, keys, values, key_lengths_i)
    return out
